# revision 15
# baseline (speedup 1.0000x reference)
"""Multi-head attention (B=4, T=2048, D=1024, H=16) on 8 TRN2 NeuronCores.

Sharding: core c -> (batch b = c//2, head-group g = c%2 of 8 heads).
Each core computes the qkv projection for its batch restricted to its 8
heads, full attention for those heads, and a partial output projection
(ctx_local @ Wout[rows of its heads]).  Host sums the two partials per batch.

All device inputs are pre-cast to bf16 on the host.  Per-core kernel,
organized so ACT (softmax exp) starts ~20us in and stays saturated:

  load wq/wk/wv/wout (bf16)
  qk-projection for head-pair 0, then v-projection (all heads),
  then for each head-pair hc: attention for both heads over all query
  quarters (S pairs = two row-tiled concurrent matmuls, one per head;
  exp on ACT [128,1024] PSUM->SBUF; ctx^T+sumexp via [v|1].T @ P;
  normalization via DVE reciprocal + gpsimd partition broadcast),
  interleaved with the qk-projection of the next pair; during the last
  pair, the output projection runs per query quarter.
"""

import numpy as np
import ml_dtypes
from contextlib import ExitStack

import concourse.bass as bass
import concourse.bacc as bacc
import concourse.tile as tile
from concourse import mybir
from concourse.bass_utils import run_bass_kernel_spmd

FP32 = mybir.dt.float32
BF16 = mybir.dt.bfloat16
EXP = mybir.ActivationFunctionType.Exp

D = 1024
T = 2048
HPC = 8          # heads per core
FC = 8           # feature chunks of 128 (projection contraction)
TS = 4           # token spans of 512
KC = 16          # k chunks of 128
QQ = 4           # query quarters of 512


def _norm(nc, rpool, ctx_sb, ctxp, hh, hc, qsl):
    """ctx_sb[hb:hb+64, hc, qsl] = ctxp[0:64] / ctxp[64] (sumexp row)."""
    hb = (hh % 2) * 64
    rtmp = rpool.tile([1, 512], FP32, tag="rtmp")
    nc.vector.tensor_copy(out=rtmp[:], in_=ctxp[64:65, :])
    rt = rpool.tile([1, 512], FP32, tag="rt")
    nc.vector.reciprocal_approx_fast(out=rt[:], in_=rtmp[:])
    rb = rpool.tile([64, 512], FP32, tag="rb")
    nc.gpsimd.partition_broadcast(rb[:], rt[0:1, :], channels=64)
    nc.vector.tensor_mul(ctx_sb[hb:hb + 64, hc, qsl], ctxp[0:64, :], rb[:])


def _qk_proj(nc, xrp, ps, xt_d, wq_sb, wk_sb, qT, kT, hc):
    """qT/kT[:, hc, :] for head pair hc: out [dims 128, tok 512] per span."""
    for ts in range(TS):
        tsl = slice(ts * 512, (ts + 1) * 512)
        xts = []
        for fc in range(FC):
            xr = xrp.tile([128, 512], BF16, tag="xr")
            nc.sync.dma_start(out=xr[:], in_=xt_d[fc * 128:(fc + 1) * 128, tsl])
            xts.append(xr)
        for w_sb, dst in ((wq_sb, qT), (wk_sb, kT)):
            p = ps.tile([128, 512], FP32, tag="proj")
            for fc in range(FC):
                nc.tensor.matmul(
                    p[:],
                    lhsT=w_sb[:, fc, hc * 128:(hc + 1) * 128],
                    rhs=xts[fc][:],
                    start=(fc == 0), stop=(fc == FC - 1))
            nc.vector.tensor_copy(out=dst[:, hc % 2, tsl], in_=p[:])


def _attention(nc, ppool, spsum, cpsum, rpool, qT, kT, v_sb, ctx_sb, hc, qq,
               extra=None):
    """Both heads of pair hc for query quarter qq.

    ``extra`` (called once per k-chunk) interleaves other PE work (the fused
    v-projection, the next pair's qk-projection, the output projection) into
    the ACT-bound attention stream."""
    qsl = slice(qq * 512, (qq + 1) * 512)
    P2 = ppool.tile([128, KC, 2, 512], BF16, tag="P2")
    ctxA = cpsum.tile([128, 512], FP32, tag="ctx")
    ctxB = cpsum.tile([128, 512], FP32, tag="ctx")
    def emit_av(kc):
        for i, ctxp in ((0, ctxA), (1, ctxB)):
            nc.tensor.matmul(
                ctxp[:],
                lhsT=v_sb[:, kc, 2 * hc + i, :],
                rhs=P2[:, kc, i, :],
                start=(kc == 0), stop=(kc == KC - 1))

    for kc in range(KC):
        sps = spsum.tile([128, 2, 512], FP32, tag="S")
        for i in range(2):          # head A on rows 0-63, head B on 64-127
            b0 = i * 64
            nc.tensor.matmul(
                sps[:, i, :],
                lhsT=kT[b0:b0 + 64, hc % 2, kc * 128:(kc + 1) * 128],
                rhs=qT[b0:b0 + 64, hc % 2, qsl],
                start=True, stop=True)
        nc.scalar.activation(
            out=P2[:, kc, :, :], in_=sps[:, :, :], func=EXP, scale=0.125)
        if extra is not None:
            extra(kc)
        # software pipeline: AV deferred one chunk so the next S pair is not
        # stuck behind it in the in-order PE stream
        if kc > 0:
            emit_av(kc - 1)
    emit_av(KC - 1)
    _norm(nc, rpool, ctx_sb, ctxA, 2 * hc, hc, qsl)
    _norm(nc, rpool, ctx_sb, ctxB, 2 * hc + 1, hc, qsl)


def _body(ctx, nc, tc, xt_d, wq_d, wk_d, wv_d, wo_d, out_d):
    persist = ctx.enter_context(tc.tile_pool(name="persist", bufs=1))
    qT = persist.tile([128, 2, T], BF16, tag="qT")
    kT = persist.tile([128, 2, T], BF16, tag="kT")
    v_sb = persist.tile([128, KC, HPC, 128], BF16, tag="v")
    ctx_sb = persist.tile([128, 4, T], BF16, tag="ctx")
    wo_sb = persist.tile([128, 4, D], BF16, tag="wo")

    nc.vector.memset(v_sb[:, :, :, 65:128], 0.0)
    nc.vector.memset(v_sb[:, :, :, 64:65], 1.0)

    wqk = ctx.enter_context(tc.tile_pool(name="wqk", bufs=1))
    wq_sb = wqk.tile([128, FC, 512], BF16, tag="wq")
    wk_sb = wqk.tile([128, FC, 512], BF16, tag="wk")
    for fc in range(FC):
        nc.sync.dma_start(out=wq_sb[:, fc, :], in_=wq_d[fc * 128:(fc + 1) * 128, :])
        nc.sync.dma_start(out=wk_sb[:, fc, :], in_=wk_d[fc * 128:(fc + 1) * 128, :])
    for cc in range(4):
        nc.sync.dma_start(out=wo_sb[:, cc, :], in_=wo_d[cc * 128:(cc + 1) * 128, :])

    xrp = ctx.enter_context(tc.tile_pool(name="xr", bufs=10))
    ps = ctx.enter_context(tc.tile_pool(name="proj", bufs=2, space="PSUM"))

    # wv stays resident through the fused v-projection (released after pair 0
    # qq 0); allocated before the attention pools.
    wvp = ctx.enter_context(tc.tile_pool(name="wv", bufs=1))
    wv_sb = wvp.tile([128, FC, 512], BF16, tag="wv")
    for fc in range(FC):
        nc.sync.dma_start(
            out=wv_sb[:, fc, :], in_=wv_d[fc * 128:(fc + 1) * 128, :])
    # qk-projection for pair 0 first so attention can start ASAP
    _qk_proj(nc, xrp, ps, xt_d, wq_sb, wk_sb, qT, kT, 0)

    with tc.tile_pool(name="P", bufs=2) as ppool, \
         tc.tile_pool(name="spsum", bufs=2, space="PSUM") as spsum, \
         tc.tile_pool(name="cpsum", bufs=2, space="PSUM") as cpsum, \
         tc.tile_pool(name="rpool", bufs=2) as rpool, \
         tc.tile_pool(name="osb", bufs=3) as osb:

        vstate = {"xts": None}

        def vproj(kc):
            if kc % 4 == 0:
                xts = []
                for fc in range(FC):
                    xr = xrp.tile([128, 512], BF16, tag="xr")
                    nc.sync.dma_start(
                        out=xr[:],
                        in_=xt_d[fc * 128:(fc + 1) * 128,
                                 (kc // 4) * 512:(kc // 4 + 1) * 512])
                    xts.append(xr)
            xts = vstate["xts"] if kc % 4 else xts
            if kc % 4 == 0:
                vstate["xts"] = xts
            psv = ps.tile([128, 512], FP32, tag="proj")
            for fc in range(FC):
                nc.tensor.matmul(
                    psv[:],
                    lhsT=xts[fc][:, (kc % 4) * 128:(kc % 4 + 1) * 128],
                    rhs=wv_sb[:, fc, :],
                    start=(fc == 0), stop=(fc == FC - 1))
            for hh in range(HPC):
                nc.vector.tensor_copy(
                    out=v_sb[:, kc, hh, 0:64],
                    in_=psv[:, hh * 64:(hh + 1) * 64])

        def make_qk_steps(next_hc):
            """64 generator steps: one fc-accumulation matmul per step of the
            next pair's qk projection (4 spans x {q,k} x 8 fc)."""
            st = {"xts": None, "p": None}

            def step(s):
                unit, fc = divmod(s, FC)
                ts, qk = divmod(unit, 2)
                tsl = slice(ts * 512, (ts + 1) * 512)
                if fc == 0 and qk == 0:
                    xts = []
                    for f2 in range(FC):
                        xr = xrp.tile([128, 512], BF16, tag="xr")
                        nc.sync.dma_start(
                            out=xr[:], in_=xt_d[f2 * 128:(f2 + 1) * 128, tsl])
                        xts.append(xr)
                    st["xts"] = xts
                w_sb, dst = ((wq_sb, qT), (wk_sb, kT))[qk]
                if fc == 0:
                    st["p"] = ps.tile([128, 512], FP32, tag="proj", name="qkp")
                nc.tensor.matmul(
                    st["p"][:],
                    lhsT=w_sb[:, fc, next_hc * 128:(next_hc + 1) * 128],
                    rhs=st["xts"][fc][:],
                    start=(fc == 0), stop=(fc == FC - 1))
                if fc == FC - 1:
                    nc.vector.tensor_copy(out=dst[:, next_hc % 2, tsl], in_=st["p"][:])
            return step

        def make_op_steps(qq_prev):
            """16 steps emitting the output projection of qq_prev's tokens
            (4 token chunks x 2 column halves x accumulate 4 cc)."""
            st = {"po": None, "ot": None}

            def step(s):
                unit, half = divmod(s, 2)
                tcg = qq_prev * 4 + unit // 2
                j2 = unit % 2
                if half == 0:
                    if j2 == 0:
                        st["ot"] = osb.tile([128, D], FP32, tag="ot", name="ot")
                    st["po"] = ps.tile([128, 512], FP32, tag="proj", name="po")
                    ccs = (0, 1)
                else:
                    ccs = (2, 3)
                for cc in ccs:
                    nc.tensor.matmul(
                        st["po"][:],
                        lhsT=ctx_sb[:, cc, tcg * 128:(tcg + 1) * 128],
                        rhs=wo_sb[:, cc, j2 * 512:(j2 + 1) * 512],
                        start=(cc == 0), stop=(cc == 3))
                if half == 1:
                    nc.vector.tensor_copy(
                        out=st["ot"][:, j2 * 512:(j2 + 1) * 512], in_=st["po"][:])
                    if j2 == 1:
                        nc.sync.dma_start(
                            out=out_d[tcg * 128:(tcg + 1) * 128, :],
                            in_=st["ot"][:])
            return step

        for hc in range(4):
            qk_step = make_qk_steps(hc + 1) if hc < 3 else None
            for qq in range(QQ):
                if hc == 0 and qq == 0:
                    extra = vproj
                elif qk_step is not None and qq in (1, 2):
                    base = (qq - 1) * 32

                    def extra(kc, base=base, qk_step=qk_step):
                        qk_step(base + 2 * kc)
                        qk_step(base + 2 * kc + 1)
                elif hc == 3 and qq >= 1:
                    op_step = make_op_steps(qq - 1)

                    def extra(kc, op_step=op_step):
                        if kc < 16:
                            op_step(kc)
                else:
                    extra = None
                _attention(nc, ppool, spsum, cpsum, rpool,
                           qT, kT, v_sb, ctx_sb, hc, qq, extra=extra)
        # tail: output projection for the last quarter
        op_step = make_op_steps(3)
        for s in range(16):
            op_step(s)


def build():
    nc = bacc.Bacc("TRN2", target_bir_lowering=False, debug=False, num_devices=8)
    xt_d = nc.dram_tensor("xt", [D, T], BF16, kind="ExternalInput").ap()
    wq_d = nc.dram_tensor("wq", [D, 512], BF16, kind="ExternalInput").ap()
    wk_d = nc.dram_tensor("wk", [D, 512], BF16, kind="ExternalInput").ap()
    wv_d = nc.dram_tensor("wv", [D, 512], BF16, kind="ExternalInput").ap()
    wo_d = nc.dram_tensor("wout", [512, D], BF16, kind="ExternalInput").ap()
    out_d = nc.dram_tensor("out", [T, D], FP32, kind="ExternalOutput").ap()
    with tile.TileContext(nc) as tc:
        with ExitStack() as ctx:
            _body(ctx, nc, tc, xt_d, wq_d, wk_d, wv_d, wo_d, out_d)
    nc.compile()
    return nc


_nc = None


def _get_nc():
    global _nc
    if _nc is None:
        _nc = build()
    return _nc


def make_in_maps(x, Wqkv, Wout):
    bf = ml_dtypes.bfloat16
    in_maps = []
    for c in range(8):
        b, g = divmod(c, 2)
        cs = slice(g * 512, (g + 1) * 512)
        in_maps.append({
            "xt": np.ascontiguousarray(x[b].T).astype(bf),
            "wq": np.ascontiguousarray(Wqkv[:, 0 * D:1 * D][:, cs]).astype(bf),
            "wk": np.ascontiguousarray(Wqkv[:, 1 * D:2 * D][:, cs]).astype(bf),
            "wv": np.ascontiguousarray(Wqkv[:, 2 * D:3 * D][:, cs]).astype(bf),
            "wout": np.ascontiguousarray(Wout[cs, :]).astype(bf),
        })
    return in_maps


def kernel(x, Wqkv, Wout, _trace=False):
    nc = _get_nc()
    x = np.asarray(x, dtype=np.float32)
    Wqkv = np.asarray(Wqkv, dtype=np.float32)
    Wout = np.asarray(Wout, dtype=np.float32)
    in_maps = make_in_maps(x, Wqkv, Wout)
    kwargs = {}
    if _trace:
        kwargs["trace"] = True
    res = run_bass_kernel_spmd(nc, in_maps, core_ids=list(range(8)), **kwargs)
    outs = [res.results[c]["out"] for c in range(8)]
    out = np.stack([outs[2 * b] + outs[2 * b + 1] for b in range(4)])
    if _trace:
        kernel.last_result = res
    return out


# revision 16
# speedup vs baseline: 1.0145x; 1.0145x over previous
"""Multi-head attention (B=4, T=2048, D=1024, H=16) on 8 TRN2 NeuronCores.

Sharding: core c -> (batch b = c//2, head-group g = c%2 of 8 heads).
Each core computes the qkv projection for its batch restricted to its 8
heads, full attention for those heads, and a partial output projection
(ctx_local @ Wout[rows of its heads]).  Host sums the two partials per batch.

All device inputs are pre-cast to bf16 on the host.  Per-core kernel,
organized so ACT (softmax exp) starts ~20us in and stays saturated:

  load wq/wk/wv/wout (bf16)
  qk-projection for head-pair 0, then v-projection (all heads),
  then for each head-pair hc: attention for both heads over all query
  quarters (S pairs = two row-tiled concurrent matmuls, one per head;
  exp on ACT [128,1024] PSUM->SBUF; ctx^T+sumexp via [v|1].T @ P;
  normalization via DVE reciprocal + gpsimd partition broadcast),
  interleaved with the qk-projection of the next pair; during the last
  pair, the output projection runs per query quarter.
"""

import numpy as np
import ml_dtypes
from contextlib import ExitStack

import concourse.bass as bass
import concourse.bacc as bacc
import concourse.tile as tile
from concourse import mybir
from concourse.bass_utils import run_bass_kernel_spmd

FP32 = mybir.dt.float32
BF16 = mybir.dt.bfloat16
EXP = mybir.ActivationFunctionType.Exp

D = 1024
T = 2048
HPC = 8          # heads per core
FC = 8           # feature chunks of 128 (projection contraction)
TS = 4           # token spans of 512
KC = 16          # k chunks of 128
QQ = 4           # query quarters of 512


def _norm(nc, rpool, ctx_sb, ctxp, hh, hc, qsl):
    """ctx_sb[hb:hb+64, hc, qsl] = ctxp[0:64] / ctxp[64] (sumexp row)."""
    hb = (hh % 2) * 64
    rtmp = rpool.tile([1, 512], FP32, tag="rtmp")
    nc.vector.tensor_copy(out=rtmp[:], in_=ctxp[64:65, :])
    rt = rpool.tile([1, 512], FP32, tag="rt")
    nc.vector.reciprocal_approx_fast(out=rt[:], in_=rtmp[:])
    rb = rpool.tile([64, 512], FP32, tag="rb")
    nc.gpsimd.partition_broadcast(rb[:], rt[0:1, :], channels=64)
    nc.vector.tensor_mul(ctx_sb[hb:hb + 64, hc, qsl], ctxp[0:64, :], rb[:])


def _qk_proj(nc, xrp, ps, xt_d, wq_sb, wk_sb, qT, kT, hc):
    """qT/kT[:, hc, :] for head pair hc: out [dims 128, tok 512] per span."""
    for ts in range(TS):
        tsl = slice(ts * 512, (ts + 1) * 512)
        xts = []
        for fc in range(FC):
            xr = xrp.tile([128, 512], BF16, tag="xr")
            nc.sync.dma_start(out=xr[:], in_=xt_d[fc * 128:(fc + 1) * 128, tsl])
            xts.append(xr)
        for w_sb, dst in ((wq_sb, qT), (wk_sb, kT)):
            p = ps.tile([128, 512], FP32, tag="proj")
            for fc in range(FC):
                nc.tensor.matmul(
                    p[:],
                    lhsT=w_sb[:, fc, hc * 128:(hc + 1) * 128],
                    rhs=xts[fc][:],
                    start=(fc == 0), stop=(fc == FC - 1))
            nc.vector.tensor_copy(out=dst[:, hc % 2, tsl], in_=p[:])


def _attention(nc, ppool, spsum, cpsum, rpool, qT, kT, v_sb, ctx_sb, hc, qq,
               extra=None):
    """Both heads of pair hc for query quarter qq.

    ``extra`` (called once per k-chunk) interleaves other PE work (the fused
    v-projection, the next pair's qk-projection, the output projection) into
    the ACT-bound attention stream."""
    qsl = slice(qq * 512, (qq + 1) * 512)
    P2 = ppool.tile([128, KC, 2, 512], BF16, tag="P2")
    ctxA = cpsum.tile([65, 512], FP32, tag="ctx")
    ctxB = cpsum.tile([65, 512], FP32, tag="ctx")
    def emit_av(kc):
        for i, ctxp in ((0, ctxA), (1, ctxB)):
            nc.tensor.matmul(
                ctxp[:],
                lhsT=v_sb[:, kc, 2 * hc + i, :],
                rhs=P2[:, kc, i, :],
                start=(kc == 0), stop=(kc == KC - 1))

    for kc in range(KC):
        sps = spsum.tile([128, 2, 512], FP32, tag="S")
        for i in range(2):          # head A on rows 0-63, head B on 64-127
            b0 = i * 64
            nc.tensor.matmul(
                sps[:, i, :],
                lhsT=kT[b0:b0 + 64, hc % 2, kc * 128:(kc + 1) * 128],
                rhs=qT[b0:b0 + 64, hc % 2, qsl],
                start=True, stop=True)
        nc.scalar.activation(
            out=P2[:, kc, :, :], in_=sps[:, :, :], func=EXP, scale=0.125)
        if extra is not None:
            extra(kc)
        # software pipeline: AV deferred one chunk so the next S pair is not
        # stuck behind it in the in-order PE stream
        if kc > 0:
            emit_av(kc - 1)
    emit_av(KC - 1)
    _norm(nc, rpool, ctx_sb, ctxA, 2 * hc, hc, qsl)
    _norm(nc, rpool, ctx_sb, ctxB, 2 * hc + 1, hc, qsl)


def _body(ctx, nc, tc, xt_d, wq_d, wk_d, wv_d, wo_d, out_d):
    persist = ctx.enter_context(tc.tile_pool(name="persist", bufs=1))
    qT = persist.tile([128, 2, T], BF16, tag="qT")
    kT = persist.tile([128, 2, T], BF16, tag="kT")
    v_sb = persist.tile([128, KC, HPC, 65], BF16, tag="v")
    ctx_sb = persist.tile([128, 4, T], BF16, tag="ctx")
    wo_sb = persist.tile([128, 4, D], BF16, tag="wo")

    nc.vector.memset(v_sb[:, :, :, 64:65], 1.0)

    wqk = ctx.enter_context(tc.tile_pool(name="wqk", bufs=1))
    wq_sb = wqk.tile([128, FC, 512], BF16, tag="wq")
    wk_sb = wqk.tile([128, FC, 512], BF16, tag="wk")
    for fc in range(FC):
        nc.sync.dma_start(out=wq_sb[:, fc, :], in_=wq_d[fc * 128:(fc + 1) * 128, :])
        nc.sync.dma_start(out=wk_sb[:, fc, :], in_=wk_d[fc * 128:(fc + 1) * 128, :])
    for cc in range(4):
        nc.sync.dma_start(out=wo_sb[:, cc, :], in_=wo_d[cc * 128:(cc + 1) * 128, :])

    xrp = ctx.enter_context(tc.tile_pool(name="xr", bufs=10))
    ps = ctx.enter_context(tc.tile_pool(name="proj", bufs=2, space="PSUM"))

    # wv stays resident through the fused v-projection (released after pair 0
    # qq 0); allocated before the attention pools.
    wvp = ctx.enter_context(tc.tile_pool(name="wv", bufs=1))
    wv_sb = wvp.tile([128, FC, 512], BF16, tag="wv")
    for fc in range(FC):
        nc.sync.dma_start(
            out=wv_sb[:, fc, :], in_=wv_d[fc * 128:(fc + 1) * 128, :])
    # qk-projection for pair 0 first so attention can start ASAP
    _qk_proj(nc, xrp, ps, xt_d, wq_sb, wk_sb, qT, kT, 0)

    with tc.tile_pool(name="P", bufs=2) as ppool, \
         tc.tile_pool(name="spsum", bufs=2, space="PSUM") as spsum, \
         tc.tile_pool(name="cpsum", bufs=2, space="PSUM") as cpsum, \
         tc.tile_pool(name="rpool", bufs=2) as rpool, \
         tc.tile_pool(name="osb", bufs=3) as osb:

        vstate = {"xts": None}

        def vproj(kc):
            if kc % 4 == 0:
                xts = []
                for fc in range(FC):
                    xr = xrp.tile([128, 512], BF16, tag="xr")
                    nc.sync.dma_start(
                        out=xr[:],
                        in_=xt_d[fc * 128:(fc + 1) * 128,
                                 (kc // 4) * 512:(kc // 4 + 1) * 512])
                    xts.append(xr)
            xts = vstate["xts"] if kc % 4 else xts
            if kc % 4 == 0:
                vstate["xts"] = xts
            psv = ps.tile([128, 512], FP32, tag="proj")
            for fc in range(FC):
                nc.tensor.matmul(
                    psv[:],
                    lhsT=xts[fc][:, (kc % 4) * 128:(kc % 4 + 1) * 128],
                    rhs=wv_sb[:, fc, :],
                    start=(fc == 0), stop=(fc == FC - 1))
            for hh in range(HPC):
                nc.vector.tensor_copy(
                    out=v_sb[:, kc, hh, 0:64],
                    in_=psv[:, hh * 64:(hh + 1) * 64])

        def make_qk_steps(next_hc):
            """64 generator steps: one fc-accumulation matmul per step of the
            next pair's qk projection (4 spans x {q,k} x 8 fc)."""
            st = {"xts": None, "p": None}

            def step(s):
                unit, fc = divmod(s, FC)
                ts, qk = divmod(unit, 2)
                tsl = slice(ts * 512, (ts + 1) * 512)
                if fc == 0 and qk == 0:
                    xts = []
                    for f2 in range(FC):
                        xr = xrp.tile([128, 512], BF16, tag="xr")
                        nc.sync.dma_start(
                            out=xr[:], in_=xt_d[f2 * 128:(f2 + 1) * 128, tsl])
                        xts.append(xr)
                    st["xts"] = xts
                w_sb, dst = ((wq_sb, qT), (wk_sb, kT))[qk]
                if fc == 0:
                    st["p"] = ps.tile([128, 512], FP32, tag="proj", name="qkp")
                nc.tensor.matmul(
                    st["p"][:],
                    lhsT=w_sb[:, fc, next_hc * 128:(next_hc + 1) * 128],
                    rhs=st["xts"][fc][:],
                    start=(fc == 0), stop=(fc == FC - 1))
                if fc == FC - 1:
                    nc.vector.tensor_copy(out=dst[:, next_hc % 2, tsl], in_=st["p"][:])
            return step

        def make_op_steps(qq_prev):
            """16 steps emitting the output projection of qq_prev's tokens
            (4 token chunks x 2 column halves x accumulate 4 cc)."""
            st = {"po": None, "ot": None}

            def step(s):
                unit, half = divmod(s, 2)
                tcg = qq_prev * 4 + unit // 2
                j2 = unit % 2
                if half == 0:
                    if j2 == 0:
                        st["ot"] = osb.tile([128, D], FP32, tag="ot", name="ot")
                    st["po"] = ps.tile([128, 512], FP32, tag="proj", name="po")
                    ccs = (0, 1)
                else:
                    ccs = (2, 3)
                for cc in ccs:
                    nc.tensor.matmul(
                        st["po"][:],
                        lhsT=ctx_sb[:, cc, tcg * 128:(tcg + 1) * 128],
                        rhs=wo_sb[:, cc, j2 * 512:(j2 + 1) * 512],
                        start=(cc == 0), stop=(cc == 3))
                if half == 1:
                    nc.vector.tensor_copy(
                        out=st["ot"][:, j2 * 512:(j2 + 1) * 512], in_=st["po"][:])
                    if j2 == 1:
                        nc.sync.dma_start(
                            out=out_d[tcg * 128:(tcg + 1) * 128, :],
                            in_=st["ot"][:])
            return step

        for hc in range(4):
            qk_step = make_qk_steps(hc + 1) if hc < 3 else None
            for qq in range(QQ):
                if hc == 0 and qq == 0:
                    extra = vproj
                elif qk_step is not None and qq in (1, 2):
                    base = (qq - 1) * 32

                    def extra(kc, base=base, qk_step=qk_step):
                        qk_step(base + 2 * kc)
                        qk_step(base + 2 * kc + 1)
                elif hc == 3 and qq >= 1:
                    op_step = make_op_steps(qq - 1)

                    def extra(kc, op_step=op_step):
                        if kc < 16:
                            op_step(kc)
                else:
                    extra = None
                _attention(nc, ppool, spsum, cpsum, rpool,
                           qT, kT, v_sb, ctx_sb, hc, qq, extra=extra)
        # tail: output projection for the last quarter
        op_step = make_op_steps(3)
        for s in range(16):
            op_step(s)


def build():
    nc = bacc.Bacc("TRN2", target_bir_lowering=False, debug=False, num_devices=8)
    xt_d = nc.dram_tensor("xt", [D, T], BF16, kind="ExternalInput").ap()
    wq_d = nc.dram_tensor("wq", [D, 512], BF16, kind="ExternalInput").ap()
    wk_d = nc.dram_tensor("wk", [D, 512], BF16, kind="ExternalInput").ap()
    wv_d = nc.dram_tensor("wv", [D, 512], BF16, kind="ExternalInput").ap()
    wo_d = nc.dram_tensor("wout", [512, D], BF16, kind="ExternalInput").ap()
    out_d = nc.dram_tensor("out", [T, D], FP32, kind="ExternalOutput").ap()
    with tile.TileContext(nc) as tc:
        with ExitStack() as ctx:
            _body(ctx, nc, tc, xt_d, wq_d, wk_d, wv_d, wo_d, out_d)
    nc.compile()
    return nc


_nc = None


def _get_nc():
    global _nc
    if _nc is None:
        _nc = build()
    return _nc


def make_in_maps(x, Wqkv, Wout):
    bf = ml_dtypes.bfloat16
    in_maps = []
    for c in range(8):
        b, g = divmod(c, 2)
        cs = slice(g * 512, (g + 1) * 512)
        in_maps.append({
            "xt": np.ascontiguousarray(x[b].T).astype(bf),
            "wq": np.ascontiguousarray(Wqkv[:, 0 * D:1 * D][:, cs]).astype(bf),
            "wk": np.ascontiguousarray(Wqkv[:, 1 * D:2 * D][:, cs]).astype(bf),
            "wv": np.ascontiguousarray(Wqkv[:, 2 * D:3 * D][:, cs]).astype(bf),
            "wout": np.ascontiguousarray(Wout[cs, :]).astype(bf),
        })
    return in_maps


def kernel(x, Wqkv, Wout, _trace=False):
    nc = _get_nc()
    x = np.asarray(x, dtype=np.float32)
    Wqkv = np.asarray(Wqkv, dtype=np.float32)
    Wout = np.asarray(Wout, dtype=np.float32)
    in_maps = make_in_maps(x, Wqkv, Wout)
    kwargs = {}
    if _trace:
        kwargs["trace"] = True
    res = run_bass_kernel_spmd(nc, in_maps, core_ids=list(range(8)), **kwargs)
    outs = [res.results[c]["out"] for c in range(8)]
    out = np.stack([outs[2 * b] + outs[2 * b + 1] for b in range(4)])
    if _trace:
        kernel.last_result = res
    return out


# revision 17
# speedup vs baseline: 1.1634x; 1.1468x over previous
"""Multi-head attention (B=4, T=2048, D=1024, H=16) on 8 TRN2 NeuronCores.

Sharding: core c -> (batch b = c//2, head-group g = c%2 of 8 heads).
Each core computes the qkv projection for its batch restricted to its 8
heads, full attention for those heads, and a partial output projection
(ctx_local @ Wout[rows of its heads]).  Host sums the two partials per batch.

All device inputs are pre-cast to bf16 on the host.  Per-core kernel,
organized so ACT (softmax exp) starts ~20us in and stays saturated:

  load wq/wk/wv/wout (bf16)
  qk-projection for head-pair 0, then v-projection (all heads),
  then for each head-pair hc: attention for both heads over all query
  quarters (S pairs = two row-tiled concurrent matmuls, one per head;
  exp on ACT [128,1024] PSUM->SBUF; ctx^T+sumexp via [v|1].T @ P;
  normalization via DVE reciprocal + gpsimd partition broadcast),
  interleaved with the qk-projection of the next pair; during the last
  pair, the output projection runs per query quarter.
"""

import numpy as np
import ml_dtypes
from contextlib import ExitStack

import concourse.bass as bass
import concourse.bacc as bacc
import concourse.tile as tile
from concourse import mybir
from concourse.bass_utils import run_bass_kernel_spmd

FP32 = mybir.dt.float32
BF16 = mybir.dt.bfloat16
EXP = mybir.ActivationFunctionType.Exp

D = 1024
T = 2048
HPC = 8          # heads per core
FC = 8           # feature chunks of 128 (projection contraction)
TS = 4           # token spans of 512
KC = 16          # k chunks of 128
QQ = 4           # query quarters of 512


def _norm(nc, rpool, ctx_sb, ctxp, hh, hc, qsl):
    """ctx_sb[hb:hb+64, hc, qsl] = ctxp[0:64] / ctxp[64] (sumexp row)."""
    hb = (hh % 2) * 64
    rtmp = rpool.tile([1, 512], FP32, tag="rtmp")
    nc.vector.tensor_copy(out=rtmp[:], in_=ctxp[64:65, :])
    rt = rpool.tile([1, 512], FP32, tag="rt")
    nc.vector.reciprocal_approx_fast(out=rt[:], in_=rtmp[:])
    rb = rpool.tile([64, 512], FP32, tag="rb")
    nc.gpsimd.partition_broadcast(rb[:], rt[0:1, :], channels=64)
    nc.vector.tensor_mul(ctx_sb[hb:hb + 64, hc, qsl], ctxp[0:64, :], rb[:])


def _qk_proj(nc, xrp, ps, xt_d, wq_sb, wk_sb, qT, kT, hc):
    """qT/kT[:, hc, :] for head pair hc: out [dims 128, tok 512] per span."""
    for ts in range(TS):
        tsl = slice(ts * 512, (ts + 1) * 512)
        xts = []
        for fc in range(FC):
            xr = xrp.tile([128, 512], BF16, tag="xr")
            nc.sync.dma_start(out=xr[:], in_=xt_d[fc * 128:(fc + 1) * 128, tsl])
            xts.append(xr)
        for w_sb, dst in ((wq_sb, qT), (wk_sb, kT)):
            p = ps.tile([128, 512], FP32, tag="proj")
            for fc in range(FC):
                nc.tensor.matmul(
                    p[:],
                    lhsT=w_sb[:, fc, hc * 128:(hc + 1) * 128],
                    rhs=xts[fc][:],
                    start=(fc == 0), stop=(fc == FC - 1))
            nc.vector.tensor_copy(out=dst[:, hc, tsl], in_=p[:])


def _attention(nc, ppool, spsum, cpsum, rpool, qT, kT, v_sb, ctx_sb, hc, qq,
               extra=None):
    """Both heads of pair hc for query quarter qq.

    ``extra`` (called once per k-chunk) interleaves other PE work (the fused
    v-projection, the next pair's qk-projection, the output projection) into
    the ACT-bound attention stream."""
    qsl = slice(qq * 512, (qq + 1) * 512)
    P2 = ppool.tile([128, KC, 2, 512], BF16, tag="P2")
    ctxA = cpsum.tile([65, 512], FP32, tag="ctx")
    ctxB = cpsum.tile([65, 512], FP32, tag="ctx")
    def emit_av(kc):
        for i, ctxp in ((0, ctxA), (1, ctxB)):
            nc.tensor.matmul(
                ctxp[:],
                lhsT=v_sb[:, kc, 2 * hc + i, :],
                rhs=P2[:, kc, i, :],
                start=(kc == 0), stop=(kc == KC - 1))

    for kc in range(KC):
        sps = spsum.tile([128, 2, 512], FP32, tag="S")
        for i in range(2):          # head A on rows 0-63, head B on 64-127
            b0 = i * 64
            nc.tensor.matmul(
                sps[:, i, :],
                lhsT=kT[b0:b0 + 64, hc, kc * 128:(kc + 1) * 128],
                rhs=qT[b0:b0 + 64, hc, qsl],
                start=True, stop=True)
        nc.scalar.activation(
            out=P2[:, kc, :, :], in_=sps[:, :, :], func=EXP, scale=0.125)
        if extra is not None:
            extra(kc)
        # software pipeline: AV deferred one chunk so the next S pair is not
        # stuck behind it in the in-order PE stream
        if kc > 0:
            emit_av(kc - 1)
    emit_av(KC - 1)
    _norm(nc, rpool, ctx_sb, ctxA, 2 * hc, hc, qsl)
    _norm(nc, rpool, ctx_sb, ctxB, 2 * hc + 1, hc, qsl)


def _body(ctx, nc, tc, xt_d, wq_d, wk_d, wv_d, wo_d, out_d):
    persist = ctx.enter_context(tc.tile_pool(name="persist", bufs=1))
    qT = persist.tile([128, 4, T], BF16, tag="qT")
    kT = persist.tile([128, 4, T], BF16, tag="kT")
    v_sb = persist.tile([128, KC, HPC, 65], BF16, tag="v")
    ctx_sb = persist.tile([128, 4, T], BF16, tag="ctx")
    wo_sb = persist.tile([128, 4, D], BF16, tag="wo")

    nc.vector.memset(v_sb[:, :, :, 64:65], 1.0)

    wqk = ctx.enter_context(tc.tile_pool(name="wqk", bufs=1))
    wq_sb = wqk.tile([128, FC, 512], BF16, tag="wq")
    wk_sb = wqk.tile([128, FC, 512], BF16, tag="wk")
    for fc in range(FC):
        nc.sync.dma_start(out=wq_sb[:, fc, :], in_=wq_d[fc * 128:(fc + 1) * 128, :])
        nc.sync.dma_start(out=wk_sb[:, fc, :], in_=wk_d[fc * 128:(fc + 1) * 128, :])
    for cc in range(4):
        nc.sync.dma_start(out=wo_sb[:, cc, :], in_=wo_d[cc * 128:(cc + 1) * 128, :])

    xrp = ctx.enter_context(tc.tile_pool(name="xr", bufs=10))
    ps = ctx.enter_context(tc.tile_pool(name="proj", bufs=2, space="PSUM"))

    # wv stays resident through the fused v-projection (released after pair 0
    # qq 0); allocated before the attention pools.
    wvp = ctx.enter_context(tc.tile_pool(name="wv", bufs=1))
    wv_sb = wvp.tile([128, FC, 512], BF16, tag="wv")
    for fc in range(FC):
        nc.sync.dma_start(
            out=wv_sb[:, fc, :], in_=wv_d[fc * 128:(fc + 1) * 128, :])
    # qk-projection for pair 0 first so attention can start ASAP
    _qk_proj(nc, xrp, ps, xt_d, wq_sb, wk_sb, qT, kT, 0)

    with tc.tile_pool(name="P", bufs=2) as ppool, \
         tc.tile_pool(name="spsum", bufs=2, space="PSUM") as spsum, \
         tc.tile_pool(name="cpsum", bufs=2, space="PSUM") as cpsum, \
         tc.tile_pool(name="rpool", bufs=2) as rpool, \
         tc.tile_pool(name="osb", bufs=3) as osb:

        vstate = {"xts": None}

        def vproj(kc):
            if kc % 4 == 0:
                xts = []
                for fc in range(FC):
                    xr = xrp.tile([128, 512], BF16, tag="xr")
                    nc.sync.dma_start(
                        out=xr[:],
                        in_=xt_d[fc * 128:(fc + 1) * 128,
                                 (kc // 4) * 512:(kc // 4 + 1) * 512])
                    xts.append(xr)
            xts = vstate["xts"] if kc % 4 else xts
            if kc % 4 == 0:
                vstate["xts"] = xts
            psv = ps.tile([128, 512], FP32, tag="proj")
            for fc in range(FC):
                nc.tensor.matmul(
                    psv[:],
                    lhsT=xts[fc][:, (kc % 4) * 128:(kc % 4 + 1) * 128],
                    rhs=wv_sb[:, fc, :],
                    start=(fc == 0), stop=(fc == FC - 1))
            for hh in range(HPC):
                nc.vector.tensor_copy(
                    out=v_sb[:, kc, hh, 0:64],
                    in_=psv[:, hh * 64:(hh + 1) * 64])

        def make_qk_steps(next_hc):
            """64 generator steps: one fc-accumulation matmul per step of the
            next pair's qk projection (4 spans x {q,k} x 8 fc)."""
            st = {"xts": None, "p": None}

            def step(s):
                unit, fc = divmod(s, FC)
                ts, qk = divmod(unit, 2)
                tsl = slice(ts * 512, (ts + 1) * 512)
                if fc == 0 and qk == 0:
                    xts = []
                    for f2 in range(FC):
                        xr = xrp.tile([128, 512], BF16, tag="xr")
                        nc.sync.dma_start(
                            out=xr[:], in_=xt_d[f2 * 128:(f2 + 1) * 128, tsl])
                        xts.append(xr)
                    st["xts"] = xts
                w_sb, dst = ((wq_sb, qT), (wk_sb, kT))[qk]
                if fc == 0:
                    st["p"] = ps.tile([128, 512], FP32, tag="proj", name="qkp")
                nc.tensor.matmul(
                    st["p"][:],
                    lhsT=w_sb[:, fc, next_hc * 128:(next_hc + 1) * 128],
                    rhs=st["xts"][fc][:],
                    start=(fc == 0), stop=(fc == FC - 1))
                if fc == FC - 1:
                    nc.vector.tensor_copy(out=dst[:, next_hc, tsl], in_=st["p"][:])
            return step

        def make_op_steps(qq_prev):
            """16 steps emitting the output projection of qq_prev's tokens
            (4 token chunks x 2 column halves x accumulate 4 cc)."""
            st = {"po": None, "ot": None}

            def step(s):
                unit, half = divmod(s, 2)
                tcg = qq_prev * 4 + unit // 2
                j2 = unit % 2
                if half == 0:
                    if j2 == 0:
                        st["ot"] = osb.tile([128, D], FP32, tag="ot", name="ot")
                    st["po"] = ps.tile([128, 512], FP32, tag="proj", name="po")
                    ccs = (0, 1)
                else:
                    ccs = (2, 3)
                for cc in ccs:
                    nc.tensor.matmul(
                        st["po"][:],
                        lhsT=ctx_sb[:, cc, tcg * 128:(tcg + 1) * 128],
                        rhs=wo_sb[:, cc, j2 * 512:(j2 + 1) * 512],
                        start=(cc == 0), stop=(cc == 3))
                if half == 1:
                    nc.vector.tensor_copy(
                        out=st["ot"][:, j2 * 512:(j2 + 1) * 512], in_=st["po"][:])
                    if j2 == 1:
                        nc.sync.dma_start(
                            out=out_d[tcg * 128:(tcg + 1) * 128, :],
                            in_=st["ot"][:])
            return step

        for hc in range(4):
            qk_step = make_qk_steps(hc + 1) if hc < 3 else None
            for qq in range(QQ):
                if hc == 0 and qq == 0:
                    extra = vproj
                elif qk_step is not None and qq in (1, 2):
                    base = (qq - 1) * 32

                    def extra(kc, base=base, qk_step=qk_step):
                        qk_step(base + 2 * kc)
                        qk_step(base + 2 * kc + 1)
                elif hc == 3 and qq >= 1:
                    op_step = make_op_steps(qq - 1)

                    def extra(kc, op_step=op_step):
                        if kc < 16:
                            op_step(kc)
                else:
                    extra = None
                _attention(nc, ppool, spsum, cpsum, rpool,
                           qT, kT, v_sb, ctx_sb, hc, qq, extra=extra)
        # tail: output projection for the last quarter
        op_step = make_op_steps(3)
        for s in range(16):
            op_step(s)


def build():
    nc = bacc.Bacc("TRN2", target_bir_lowering=False, debug=False, num_devices=8)
    xt_d = nc.dram_tensor("xt", [D, T], BF16, kind="ExternalInput").ap()
    wq_d = nc.dram_tensor("wq", [D, 512], BF16, kind="ExternalInput").ap()
    wk_d = nc.dram_tensor("wk", [D, 512], BF16, kind="ExternalInput").ap()
    wv_d = nc.dram_tensor("wv", [D, 512], BF16, kind="ExternalInput").ap()
    wo_d = nc.dram_tensor("wout", [512, D], BF16, kind="ExternalInput").ap()
    out_d = nc.dram_tensor("out", [T, D], FP32, kind="ExternalOutput").ap()
    with tile.TileContext(nc) as tc:
        with ExitStack() as ctx:
            _body(ctx, nc, tc, xt_d, wq_d, wk_d, wv_d, wo_d, out_d)
    nc.compile()
    return nc


_nc = None


def _get_nc():
    global _nc
    if _nc is None:
        _nc = build()
    return _nc


def make_in_maps(x, Wqkv, Wout):
    bf = ml_dtypes.bfloat16
    in_maps = []
    for c in range(8):
        b, g = divmod(c, 2)
        cs = slice(g * 512, (g + 1) * 512)
        in_maps.append({
            "xt": np.ascontiguousarray(x[b].T).astype(bf),
            "wq": np.ascontiguousarray(Wqkv[:, 0 * D:1 * D][:, cs]).astype(bf),
            "wk": np.ascontiguousarray(Wqkv[:, 1 * D:2 * D][:, cs]).astype(bf),
            "wv": np.ascontiguousarray(Wqkv[:, 2 * D:3 * D][:, cs]).astype(bf),
            "wout": np.ascontiguousarray(Wout[cs, :]).astype(bf),
        })
    return in_maps


def kernel(x, Wqkv, Wout, _trace=False):
    nc = _get_nc()
    x = np.asarray(x, dtype=np.float32)
    Wqkv = np.asarray(Wqkv, dtype=np.float32)
    Wout = np.asarray(Wout, dtype=np.float32)
    in_maps = make_in_maps(x, Wqkv, Wout)
    kwargs = {}
    if _trace:
        kwargs["trace"] = True
    res = run_bass_kernel_spmd(nc, in_maps, core_ids=list(range(8)), **kwargs)
    outs = [res.results[c]["out"] for c in range(8)]
    out = np.stack([outs[2 * b] + outs[2 * b + 1] for b in range(4)])
    if _trace:
        kernel.last_result = res
    return out


# revision 20
# speedup vs baseline: 1.2561x; 1.0797x over previous
"""Multi-head attention (B=4, T=2048, D=1024, H=16) on 8 TRN2 NeuronCores.

Sharding: core c -> (batch b = c//2, head-group g = c%2 of 8 heads).
Each core computes the qkv projection for its batch restricted to its 8
heads, full attention for those heads, and a partial output projection
(ctx_local @ Wout[rows of its heads]).  Host sums the two partials per batch.

All device inputs are pre-cast to bf16 on the host.  Per-core kernel,
organized so ACT (softmax exp) starts ~20us in and stays saturated:

  load wq/wk/wv/wout (bf16)
  qk-projection for head-pair 0, then v-projection (all heads),
  then for each head-pair hc: attention for both heads over all query
  quarters (S pairs = two row-tiled concurrent matmuls, one per head;
  exp on ACT [128,1024] PSUM->SBUF; ctx^T+sumexp via [v|1].T @ P;
  normalization via DVE reciprocal + gpsimd partition broadcast),
  interleaved with the qk-projection of the next pair; during the last
  pair, the output projection runs per query quarter.
"""

import numpy as np
import ml_dtypes
from contextlib import ExitStack

import concourse.bass as bass
import concourse.bacc as bacc
import concourse.tile as tile
from concourse import mybir
from concourse.bass_utils import run_bass_kernel_spmd

FP32 = mybir.dt.float32
BF16 = mybir.dt.bfloat16
EXP = mybir.ActivationFunctionType.Exp

D = 1024
T = 2048
HPC = 8          # heads per core
FC = 8           # feature chunks of 128 (projection contraction)
TS = 4           # token spans of 512
KC = 16          # k chunks of 128
QQ = 4           # query quarters of 512


def _norm(nc, rpool, ctx_sb, ctxp, hh, hc, qsl):
    """ctx_sb[hb:hb+64, hc, qsl] = ctxp[0:64] / ctxp[64] (sumexp row)."""
    hb = (hh % 2) * 64
    rtmp = rpool.tile([1, 512], FP32, tag="rtmp")
    nc.vector.tensor_copy(out=rtmp[:], in_=ctxp[64:65, :])
    rt = rpool.tile([1, 512], FP32, tag="rt")
    nc.vector.reciprocal_approx_fast(out=rt[:], in_=rtmp[:])
    rb = rpool.tile([64, 512], FP32, tag="rb")
    nc.gpsimd.partition_broadcast(rb[:], rt[0:1, :], channels=64)
    nc.vector.tensor_mul(ctx_sb[hb:hb + 64, hc, qsl], ctxp[0:64, :], rb[:])


def _qk_proj(nc, xrp, ps, xt_r, wq_sb, wk_sb, qT, kT, hc):
    """qT/kT[:, hc, :] for head pair hc: out [dims 128, tok 512] per span."""
    for ts in range(TS):
        tsl = slice(ts * 512, (ts + 1) * 512)
        xsp = xrp.tile([128, FC, 512], BF16, tag="xr")
        nc.sync.dma_start(out=xsp[:], in_=xt_r[:, :, tsl])
        xts = [xsp[:, fc, :] for fc in range(FC)]
        for w_sb, dst in ((wq_sb, qT), (wk_sb, kT)):
            p = ps.tile([128, 512], FP32, tag="proj")
            for fc in range(FC):
                nc.tensor.matmul(
                    p[:],
                    lhsT=w_sb[:, fc, hc * 128:(hc + 1) * 128],
                    rhs=xts[fc],
                    start=(fc == 0), stop=(fc == FC - 1))
            nc.vector.tensor_copy(out=dst[:, hc, tsl], in_=p[:])


def _attention(nc, ppool, spsum, cpsum, rpool, qT, kT, v_sb, ctx_sb, hc, qq,
               extra=None):
    """Both heads of pair hc for query quarter qq.

    ``extra`` (called once per k-chunk) interleaves other PE work (the fused
    v-projection, the next pair's qk-projection, the output projection) into
    the ACT-bound attention stream."""
    qsl = slice(qq * 512, (qq + 1) * 512)
    P2 = ppool.tile([128, KC, 2, 512], BF16, tag="P2")
    ctxA = cpsum.tile([65, 512], FP32, tag="ctx")
    ctxB = cpsum.tile([65, 512], FP32, tag="ctx")
    def emit_av(kc):
        for i, ctxp in ((0, ctxA), (1, ctxB)):
            nc.tensor.matmul(
                ctxp[:],
                lhsT=v_sb[:, kc, 2 * hc + i, :],
                rhs=P2[:, kc, i, :],
                start=(kc == 0), stop=(kc == KC - 1))

    for kc in range(KC):
        sps = spsum.tile([128, 2, 512], FP32, tag="S")
        for i in range(2):          # head A on rows 0-63, head B on 64-127
            b0 = i * 64
            nc.tensor.matmul(
                sps[:, i, :],
                lhsT=kT[b0:b0 + 64, hc, kc * 128:(kc + 1) * 128],
                rhs=qT[b0:b0 + 64, hc, qsl],
                start=True, stop=True)
        nc.scalar.activation(
            out=P2[:, kc, :, :], in_=sps[:, :, :], func=EXP, scale=0.125)
        if extra is not None:
            extra(kc)
        # software pipeline: AV deferred one chunk so the next S pair is not
        # stuck behind it in the in-order PE stream
        if kc > 0:
            emit_av(kc - 1)
    emit_av(KC - 1)
    _norm(nc, rpool, ctx_sb, ctxA, 2 * hc, hc, qsl)
    _norm(nc, rpool, ctx_sb, ctxB, 2 * hc + 1, hc, qsl)


def _body(ctx, nc, tc, xt_d, wq_d, wk_d, wv_d, wo_d, out_d):
    xt_r = xt_d.rearrange("(f p) t -> p f t", p=128)
    persist = ctx.enter_context(tc.tile_pool(name="persist", bufs=1))
    qT = persist.tile([128, 4, T], BF16, tag="qT")
    kT = persist.tile([128, 4, T], BF16, tag="kT")
    v_sb = persist.tile([128, KC, HPC, 65], BF16, tag="v")
    ctx_sb = persist.tile([128, 4, T], BF16, tag="ctx")
    wo_sb = persist.tile([128, 4, D], BF16, tag="wo")

    nc.vector.memset(v_sb[:, :, :, 64:65], 1.0)

    wqk = ctx.enter_context(tc.tile_pool(name="wqk", bufs=1))
    wq_sb = wqk.tile([128, FC, 512], BF16, tag="wq")
    wk_sb = wqk.tile([128, FC, 512], BF16, tag="wk")
    nc.sync.dma_start(out=wq_sb[:], in_=wq_d.rearrange("(f p) c -> p f c", p=128))
    nc.sync.dma_start(out=wk_sb[:], in_=wk_d.rearrange("(f p) c -> p f c", p=128))
    nc.sync.dma_start(out=wo_sb[:], in_=wo_d.rearrange("(c p) d -> p c d", p=128))

    xrp = ctx.enter_context(tc.tile_pool(name="xr", bufs=3))
    ps = ctx.enter_context(tc.tile_pool(name="proj", bufs=2, space="PSUM"))

    # wv stays resident through the fused v-projection (released after pair 0
    # qq 0); allocated before the attention pools.
    wvp = ctx.enter_context(tc.tile_pool(name="wv", bufs=1))
    wv_sb = wvp.tile([128, FC, 512], BF16, tag="wv")
    for fc in range(FC):
        nc.sync.dma_start(
            out=wv_sb[:, fc, :], in_=wv_d[fc * 128:(fc + 1) * 128, :])
    # qk-projection for pair 0 first so attention can start ASAP
    _qk_proj(nc, xrp, ps, xt_r, wq_sb, wk_sb, qT, kT, 0)

    with tc.tile_pool(name="P", bufs=2) as ppool, \
         tc.tile_pool(name="spsum", bufs=2, space="PSUM") as spsum, \
         tc.tile_pool(name="cpsum", bufs=2, space="PSUM") as cpsum, \
         tc.tile_pool(name="rpool", bufs=2) as rpool, \
         tc.tile_pool(name="osb", bufs=2) as osb:

        vstate = {"xts": None}

        def vproj(kc):
            if kc % 4 == 0:
                xsp = xrp.tile([128, FC, 512], BF16, tag="xr", name="xsp")
                nc.sync.dma_start(
                    out=xsp[:],
                    in_=xt_r[:, :, (kc // 4) * 512:(kc // 4 + 1) * 512])
                vstate["xts"] = xsp
            xsp = vstate["xts"]
            psv = ps.tile([128, 512], FP32, tag="proj")
            for fc in range(FC):
                nc.tensor.matmul(
                    psv[:],
                    lhsT=xsp[:, fc, (kc % 4) * 128:(kc % 4 + 1) * 128],
                    rhs=wv_sb[:, fc, :],
                    start=(fc == 0), stop=(fc == FC - 1))
            for hh in range(HPC):
                nc.vector.tensor_copy(
                    out=v_sb[:, kc, hh, 0:64],
                    in_=psv[:, hh * 64:(hh + 1) * 64])

        def make_qk_steps(next_hc):
            """64 generator steps: one fc-accumulation matmul per step of the
            next pair's qk projection (4 spans x {q,k} x 8 fc)."""
            st = {"xts": None, "p": None}

            def step(s):
                unit, fc = divmod(s, FC)
                ts, qk = divmod(unit, 2)
                tsl = slice(ts * 512, (ts + 1) * 512)
                if fc == 0 and qk == 0:
                    xsp = xrp.tile([128, FC, 512], BF16, tag="xr", name="xsp2")
                    nc.sync.dma_start(out=xsp[:], in_=xt_r[:, :, tsl])
                    st["xts"] = xsp
                w_sb, dst = ((wq_sb, qT), (wk_sb, kT))[qk]
                if fc == 0:
                    st["p"] = ps.tile([128, 512], FP32, tag="proj", name="qkp")
                nc.tensor.matmul(
                    st["p"][:],
                    lhsT=w_sb[:, fc, next_hc * 128:(next_hc + 1) * 128],
                    rhs=st["xts"][:, fc, :],
                    start=(fc == 0), stop=(fc == FC - 1))
                if fc == FC - 1:
                    nc.vector.tensor_copy(out=dst[:, next_hc, tsl], in_=st["p"][:])
            return step

        def make_op_steps(qq_prev):
            """16 steps emitting the output projection of qq_prev's tokens
            (4 token chunks x 2 column halves x accumulate 4 cc)."""
            st = {"po": None, "ot": None}

            def step(s):
                unit, half = divmod(s, 2)
                tcg = qq_prev * 4 + unit // 2
                j2 = unit % 2
                if half == 0:
                    if j2 == 0:
                        st["ot"] = osb.tile([128, D], FP32, tag="ot", name="ot")
                    st["po"] = ps.tile([128, 512], FP32, tag="proj", name="po")
                    ccs = (0, 1)
                else:
                    ccs = (2, 3)
                for cc in ccs:
                    nc.tensor.matmul(
                        st["po"][:],
                        lhsT=ctx_sb[:, cc, tcg * 128:(tcg + 1) * 128],
                        rhs=wo_sb[:, cc, j2 * 512:(j2 + 1) * 512],
                        start=(cc == 0), stop=(cc == 3))
                if half == 1:
                    nc.vector.tensor_copy(
                        out=st["ot"][:, j2 * 512:(j2 + 1) * 512], in_=st["po"][:])
                    if j2 == 1:
                        nc.sync.dma_start(
                            out=out_d[tcg * 128:(tcg + 1) * 128, :],
                            in_=st["ot"][:])
            return step

        for hc in range(4):
            qk_step = make_qk_steps(hc + 1) if hc < 3 else None
            for qq in range(QQ):
                if hc == 0 and qq == 0:
                    extra = vproj
                elif qk_step is not None and qq in (1, 2):
                    base = (qq - 1) * 32

                    def extra(kc, base=base, qk_step=qk_step):
                        qk_step(base + 2 * kc)
                        qk_step(base + 2 * kc + 1)
                elif hc == 3 and qq >= 1:
                    op_step = make_op_steps(qq - 1)

                    def extra(kc, op_step=op_step):
                        if kc < 16:
                            op_step(kc)
                else:
                    extra = None
                _attention(nc, ppool, spsum, cpsum, rpool,
                           qT, kT, v_sb, ctx_sb, hc, qq, extra=extra)
        # tail: output projection for the last quarter
        op_step = make_op_steps(3)
        for s in range(16):
            op_step(s)


def build():
    nc = bacc.Bacc("TRN2", target_bir_lowering=False, debug=False, num_devices=8)
    xt_d = nc.dram_tensor("xt", [D, T], BF16, kind="ExternalInput").ap()
    wq_d = nc.dram_tensor("wq", [D, 512], BF16, kind="ExternalInput").ap()
    wk_d = nc.dram_tensor("wk", [D, 512], BF16, kind="ExternalInput").ap()
    wv_d = nc.dram_tensor("wv", [D, 512], BF16, kind="ExternalInput").ap()
    wo_d = nc.dram_tensor("wout", [512, D], BF16, kind="ExternalInput").ap()
    out_d = nc.dram_tensor("out", [T, D], FP32, kind="ExternalOutput").ap()
    with tile.TileContext(nc) as tc:
        with ExitStack() as ctx:
            _body(ctx, nc, tc, xt_d, wq_d, wk_d, wv_d, wo_d, out_d)
    nc.compile()
    return nc


_nc = None


def _get_nc():
    global _nc
    if _nc is None:
        _nc = build()
    return _nc


def make_in_maps(x, Wqkv, Wout):
    bf = ml_dtypes.bfloat16
    in_maps = []
    for c in range(8):
        b, g = divmod(c, 2)
        cs = slice(g * 512, (g + 1) * 512)
        in_maps.append({
            "xt": np.ascontiguousarray(x[b].T).astype(bf),
            "wq": np.ascontiguousarray(Wqkv[:, 0 * D:1 * D][:, cs]).astype(bf),
            "wk": np.ascontiguousarray(Wqkv[:, 1 * D:2 * D][:, cs]).astype(bf),
            "wv": np.ascontiguousarray(Wqkv[:, 2 * D:3 * D][:, cs]).astype(bf),
            "wout": np.ascontiguousarray(Wout[cs, :]).astype(bf),
        })
    return in_maps


def kernel(x, Wqkv, Wout, _trace=False):
    nc = _get_nc()
    x = np.asarray(x, dtype=np.float32)
    Wqkv = np.asarray(Wqkv, dtype=np.float32)
    Wout = np.asarray(Wout, dtype=np.float32)
    in_maps = make_in_maps(x, Wqkv, Wout)
    kwargs = {}
    if _trace:
        kwargs["trace"] = True
    res = run_bass_kernel_spmd(nc, in_maps, core_ids=list(range(8)), **kwargs)
    outs = [res.results[c]["out"] for c in range(8)]
    out = np.stack([outs[2 * b] + outs[2 * b + 1] for b in range(4)])
    if _trace:
        kernel.last_result = res
    return out
